# revision 15
# baseline (speedup 1.0000x reference)
"""Trainium2 Bass kernel for nn_DiffAtten (diffusion GNN + multi-head attention).

Model (per batch b): qc = LN([x; Ax; A^2x]) (L=3072 rows), vc likewise with v-graph;
MHA over L with H=4 heads of dim 16; o = attn-out @ w_fc + qc; LN; pool triples of
rows; conv+relu+linear+residual; final LN.  Output [2, 1024, 64] f32.

Sharding: 8 cores = 2 batches x 4 groups.  Core (b, g) computes attention for the
L-contiguous query chunk [768g, 768(g+1)) (which exactly covers output nodes
[256g, 256(g+1)) after the triple-pooling), using the full K/V side (3072 keys)
computed on-core from the full adjacency of batch b.  No collectives; all per-core
specialisation is carried by host-side input slicing:
  - atq/atv: A^T (pre-transposed, bf16); stationary matmul operand for diffusion.
  - acq: transposed chunk-operator rows of [I; A; A^2] for the core's L-chunk.
  - m_bf: per-head M_h = (Wq_h Wk_h^T)/sqrt(dk), so scores = qc M_h qc_chunk^T.
  - wv_aug: V projection augmented with a ones row/column per head; the
    attention@V matmul then emits softmax denominators for free.

Perf notes (v2): everything the PE streams is bf16 (1 cycle/col vs 4 for f32);
score matmuls are packed 2-heads-per-matmul (512 free columns); the attention
loop is software-pipelined (scores of kt+1 issue before AV of kt so the in-order
PE never waits on the exp); LN applies / PSUM drains run on the idle Pool engine;
rsqrt is 2 Newton steps on DVE; inputs stream in per-tile so compute starts ~2us
in; the triple-pool epilogue is split so only the n=1 half waits on the last
attention third.
"""

import numpy as np

B, N, D = 2, 1024, 64
H, DK, DV = 4, 16, 16
DOUT = 128
STEPS = 3
L = STEPS * N          # 3072
P = 128
NT = N // P            # 8 node tiles
LT = L // P            # 24 L tiles
CH = L // 4            # 768 q-chunk per core
CN = N // 4            # 256 output nodes per core
QT3 = CH // 3          # 256 q columns per inner third
DV1 = DV + 1           # 17
RSQRT_MAGIC = 0x5F3759DF

_CACHE = {}


def _bcast_ap(bass_mod, ap, parts):
    """[F] dram AP -> [parts, F] broadcast AP (partition step 0)."""
    return bass_mod.AP(tensor=ap.tensor, offset=ap.offset, ap=[[0, parts]] + list(ap.ap))


def _build_nc():
    import concourse.bass as bass
    import concourse.bacc as bacc
    import concourse.tile as tile
    from concourse import mybir, masks

    f32 = mybir.dt.float32
    i32 = mybir.dt.int32
    bf16 = mybir.dt.bfloat16
    AF = mybir.ActivationFunctionType
    OP = mybir.AluOpType

    nc = bacc.Bacc(None, target_bir_lowering=False)

    # ---- kernel I/O (per-core slices supplied by the host) ----
    xqb = nc.dram_tensor("xqb", [N, D], bf16, kind="ExternalInput")
    xvb = nc.dram_tensor("xvb", [N, D], bf16, kind="ExternalInput")
    atq = nc.dram_tensor("atq", [N, N], bf16, kind="ExternalInput")
    atv = nc.dram_tensor("atv", [N, N], bf16, kind="ExternalInput")
    acq = nc.dram_tensor("acq", [N, CH], bf16, kind="ExternalInput")
    m_bf = nc.dram_tensor("m_bf", [D, H * D], bf16, kind="ExternalInput")
    wv_aug = nc.dram_tensor("wv_aug", [D + 1, H * DV1], bf16, kind="ExternalInput")
    wfc = nc.dram_tensor("wfc", [D, D], bf16, kind="ExternalInput")
    mha_w = nc.dram_tensor("mha_w", [D], f32, kind="ExternalInput")
    mha_b = nc.dram_tensor("mha_b", [D], f32, kind="ExternalInput")
    conv_w3 = nc.dram_tensor("conv_w3", [D, DOUT], bf16, kind="ExternalInput")
    conv_b = nc.dram_tensor("conv_b", [DOUT], f32, kind="ExternalInput")
    lin_w = nc.dram_tensor("lin_w", [DOUT, D], bf16, kind="ExternalInput")
    lin_b = nc.dram_tensor("lin_b", [D], f32, kind="ExternalInput")
    norm_w = nc.dram_tensor("norm_w", [D], f32, kind="ExternalInput")
    norm_b = nc.dram_tensor("norm_b", [D], f32, kind="ExternalInput")
    rest = nc.dram_tensor("rest", [D, CN], f32, kind="ExternalInput")
    out_d = nc.dram_tensor("out", [CN, D], f32, kind="ExternalOutput")

    with tile.TileContext(nc) as tc:
        with (
            tc.tile_pool(name="consts", bufs=1) as consts,
            tc.tile_pool(name="big", bufs=1) as big,
            tc.tile_pool(name="tmp", bufs=4) as tmp,
            tc.tile_pool(name="ntmp", bufs=2) as ntmp,
        ):
            # ---------------- constants ----------------
            idn = consts.tile([P, P], f32)
            masks.make_identity(nc, idn[:, :])
            idnb = consts.tile([P, P], bf16)
            masks.make_identity(nc, idnb[:, :])

            # ---- input DMAs, in consumption order.  The chunk path (xqb, acq)
            # gates everything; acq is split so z-matmuls start ~2us in.
            xqb_sb = big.tile([P, NT, D], bf16)
            nc.sync.dma_start(xqb_sb[:, :, :], xqb[:, :].rearrange("(t p) d -> p t d", p=P))
            xvb_sb = big.tile([P, NT, D], bf16)
            nc.sync.dma_start(xvb_sb[:, :, :], xvb[:, :].rearrange("(t p) d -> p t d", p=P))
            m_sb = consts.tile([P, H * D], bf16)   # padded to 128 rows (zeros below)
            nc.scalar.dma_start(m_sb[0:D, :], m_bf[:, :])
            wva_sb = consts.tile([D + 1, H * DV1], bf16)
            nc.scalar.dma_start(wva_sb[:, :], wv_aug[:, :])
            wfc_sb = consts.tile([D, D], bf16)
            nc.scalar.dma_start(wfc_sb[:, :], wfc[:, :])
            convw_sb = consts.tile([D, DOUT], bf16)
            nc.scalar.dma_start(convw_sb[:, :], conv_w3[:, :])
            convb_sb = consts.tile([DOUT, 1], f32)
            nc.scalar.dma_start(convb_sb[:, :], conv_b[:].unsqueeze(1))
            linw_sb = consts.tile([DOUT, D], bf16)
            nc.scalar.dma_start(linw_sb[:, :], lin_w[:, :])
            linb_sb = consts.tile([D, 1], f32)
            nc.scalar.dma_start(linb_sb[:, :], lin_b[:].unsqueeze(1))
            rest_sb = consts.tile([D, CN], f32)
            nc.scalar.dma_start(rest_sb[:, :], rest[:, :])
            mw_sb = consts.tile([P, D], f32)
            nc.scalar.dma_start(mw_sb[:, :], _bcast_ap(bass, mha_w[:], P))
            mb_sb = consts.tile([P, D], f32)
            nc.scalar.dma_start(mb_sb[:, :], _bcast_ap(bass, mha_b[:], P))
            nw_sb = consts.tile([P, D], f32)
            nc.scalar.dma_start(nw_sb[:, :], _bcast_ap(bass, norm_w[:], P))
            nb_sb = consts.tile([P, D], f32)
            nc.scalar.dma_start(nb_sb[:, :], _bcast_ap(bass, norm_b[:], P))
            acq_sb = big.tile([P, NT, CH], bf16)  # chunk operator^T (own DMA queue)
            for hf in range(2):
                t0 = hf * (NT // 2)
                nc.gpsimd.dma_start(
                    acq_sb[:, t0:t0 + NT // 2, :],
                    acq[t0 * P:(t0 + NT // 2) * P, :].rearrange("(t p) c -> p t c", p=P))
            # bulk adjacency on the same queue AFTER acq: the per-queue FIFO
            # keeps them from stealing DMA bandwidth from the chunk path.
            atq_sb = big.tile([P, NT, N], bf16)   # A_q^T, row jt = t*128+p
            nc.gpsimd.dma_start(atq_sb[:, :, :], atq[:, :].rearrange("(t p) i -> p t i", p=P))
            atv_sb = big.tile([P, NT, N], bf16)
            nc.gpsimd.dma_start(atv_sb[:, :, :], atv[:, :].rearrange("(t p) i -> p t i", p=P))

            # persistent intermediates
            d_rows_f = {}   # (side, step) -> [128, 8, 64] f32 rows of A^s x
            for side in ("q", "v"):
                for step in (1, 2):
                    d_rows_f[(side, step)] = big.tile(
                        [P, NT, D], f32, tag=f"d{side}{step}f", name=f"d{side}{step}f")
            dq1b = big.tile([P, NT, D], bf16)
            dv1b = big.tile([P, NT, D], bf16)

            qc_rows = big.tile([P, LT, D], bf16)   # LN'd rows (bf16, feed transposes)
            vc_rows = big.tile([P, LT, D], bf16)
            qcT_bf = big.tile([P, L], bf16)    # rows D: zero (contract-128 pad)
            vcT_bf = big.tile([D + 1, L], bf16)
            vrows_bf = big.tile([P, LT, H * DV1], bf16)
            qT_bf = big.tile([P, H, CH], bf16)
            qcTc_bf = big.tile([P, CH], bf16)       # qc chunk^T (bf16, padded)
            mv_q = big.tile([P, LT, 2], f32)
            mv_v = big.tile([P, LT, 2], f32)
            rs_q = big.tile([P, LT], f32)
            rs_v = big.tile([P, LT], f32)
            # o~^T: dim1 = head pair; heads of a pair at partition bases 0 / 64
            oT_sb = big.tile([P, 2, 3, QT3], f32)
            onr_bf = big.tile([P, 6, D], bf16)      # normalized attn out rows
            onT_bf = big.tile([D, CH], bf16)
            o2r_sb = big.tile([P, 6, D], f32)       # (o@wfc + qc) rows
            ol1_sb = big.tile([P, 6, D], f32)       # (o2 - mu) * w scratch
            oln_bf = big.tile([P, 6, D], bf16)      # after mha_ln (bf16 rows)
            zr = big.tile([P, 6, D], f32)           # chunk qc rows (f32, residual)
            zrb = big.tile([P, 6, D], bf16)
            zT_bf = big.tile([D, CH], bf16)
            mv2 = big.tile([P, 6, 2], f32)
            rst2 = big.tile([P, 6], f32)
            pool_in = big.tile([P, 2, 3 * D], bf16)
            xT_bf = big.tile([D, CN], bf16)
            x1_bf = big.tile([DOUT, CN], bf16)
            x3T_sb = big.tile([D, CN], f32)
            xr_sb = big.tile([P, 2, D], f32)
            yout = big.tile([P, 2, D], f32)
            mv3 = big.tile([P, 2, 2], f32)
            rst3 = big.tile([P, 2], f32)

            nc.gpsimd.memset(vcT_bf[D:D + 1, :], 1.0)   # ones row for V denominators
            # zero pads: the PE streams contract-64 operands at half rate, so
            # scores/qps matmuls carry 64 zero rows to keep contract=128.
            nc.gpsimd.memset(qcT_bf[D:P, :], 0.0)
            nc.gpsimd.memset(qT_bf[D:P, :, :], 0.0)
            nc.gpsimd.memset(qcTc_bf[D:P, :], 0.0)
            nc.gpsimd.memset(m_sb[D:P, :], 0.0)

            def rsqrt_newton(dst, src, shape, tag, iters=1):
                """dst = 1/sqrt(src) elementwise via fast-inverse-sqrt + Newton.
                src must be > 0. shape = [parts, free]. All on DVE.
                1 iteration: rel err ~1.7e-3; LN row-scale errors largely
                cancel (softmax is scale-covariant per row, LN renormalizes)."""
                hv = ntmp.tile(shape, f32, tag=tag + "h", name=tag + "h")
                nc.vector.tensor_scalar_mul(hv[:, :], src, 0.5)
                y = dst
                nc.vector.tensor_scalar(
                    out=y.bitcast(i32), in0=src.bitcast(i32),
                    scalar1=1, scalar2=None, op0=OP.logical_shift_right)
                nc.vector.tensor_scalar(
                    out=y.bitcast(i32), in0=y.bitcast(i32),
                    scalar1=-1, scalar2=None, op0=OP.bitwise_xor)
                nc.vector.tensor_scalar(
                    out=y.bitcast(i32), in0=y.bitcast(i32),
                    scalar1=RSQRT_MAGIC + 1, scalar2=None, op0=OP.add)
                t = ntmp.tile(shape, f32, tag=tag + "t", name=tag + "t")
                for _ in range(iters):
                    nc.vector.tensor_mul(t[:, :], y, y)
                    nc.vector.tensor_tensor(out=t[:, :], in0=t[:, :], in1=hv[:, :], op=OP.mult)
                    nc.vector.tensor_scalar(
                        out=t[:, :], in0=t[:, :], scalar1=-1.0, scalar2=1.5,
                        op0=OP.mult, op1=OP.add)
                    nc.vector.tensor_mul(y, y, t[:, :])

            def ln_group(srcs, kts, rows, mv, rs, eps, tdst, tpool, ceng=None):
                """Stats (DVE) + apply (Pool, bf16 out) + bf16 transpose (PE) for
                row tiles kts; write bf16 rows and transposed columns of tdst."""
                for i in kts:
                    st = tmp.tile([P, 6], f32, tag="bnst")
                    nc.vector.bn_stats(st[:, :], srcs[i // NT][:, i % NT, :])
                    nc.vector.bn_aggr(mv[:, i, :], st[:, :])
                i0, n = kts[0], len(kts)
                ve = tmp.tile([P, LT], f32, tag="ve")
                nc.vector.tensor_scalar_add(ve[:, i0:i0 + n], mv[:, i0:i0 + n, 1], eps)
                rsqrt_newton(rs[:, i0:i0 + n], ve[:, i0:i0 + n], [P, n], "lng")
                for i in kts:
                    nc.vector.tensor_scalar(
                        out=rows[:, i, :], in0=srcs[i // NT][:, i % NT, :],
                        scalar1=mv[:, i, 0:1], scalar2=rs[:, i:i + 1],
                        op0=OP.subtract, op1=OP.mult)
                for q0 in range(kts[0], kts[0] + len(kts), 4):
                    tpb = tpool.tile([D, 4, P], bf16, tag="tp")
                    for m in range(4):
                        nc.tensor.transpose(tpb[:, m, :], rows[:, q0 + m, :], idnb[:, :])
                    ap_dst = tdst[:D, P * q0:P * (q0 + 4)]
                    ap_src = tpb[:, :, :].rearrange("d m p -> d (m p)")
                    if ceng is nc.scalar:
                        nc.scalar.activation(ap_dst, ap_src, AF.Copy, scale=1.0)
                    else:
                        nc.vector.tensor_copy(ap_dst, ap_src)

            def v_rows(kts, tpool, ceng=None):
                for q0 in range(kts[0], kts[0] + len(kts), 4):
                    vps = tpool.tile([P, 4, H * DV1], f32, tag="tp")
                    for m in range(4):
                        nc.tensor.matmul(vps[:, m, :],
                                         lhsT=vcT_bf[:, P * (q0 + m):P * (q0 + m + 1)],
                                         rhs=wva_sb[:, :], start=True, stop=True)
                    if ceng is nc.scalar:
                        nc.scalar.activation(vrows_bf[:, q0:q0 + 4, :], vps[:, :, :],
                                             AF.Copy, scale=1.0)
                    else:
                        nc.vector.tensor_copy(vrows_bf[:, q0:q0 + 4, :], vps[:, :, :])

            def scores_iter(t3, kt, scp):
                sc = scp.tile([P, H, QT3], f32, tag="sc")
                for hp in range(2):
                    nc.tensor.matmul(
                        sc[:, 2 * hp:2 * hp + 2, :],
                        lhsT=qcT_bf[:, P * kt:P * (kt + 1)],
                        rhs=qT_bf[:, 2 * hp:2 * hp + 2, QT3 * t3:QT3 * (t3 + 1)],
                        start=True, stop=True)
                return sc

            def exp_iter(sc, expp):
                ex = expp.tile([P, H, QT3], bf16, tag="ex")
                nc.scalar.activation(ex[:, :, :], sc[:, :, :], AF.Exp, scale=1.0)
                return ex

            def av_iter(kt, ex, av_fn):
                for h in range(H):
                    nc.tensor.matmul(av_fn(h),
                                     lhsT=vrows_bf[:, kt, DV1 * h:DV1 * (h + 1)],
                                     rhs=ex[:, h, :],
                                     start=(kt == 0), stop=(kt == LT - 1),
                                     skip_group_check=True)

            def o_chain(t3, tpool, ceng=None):
                """Normalize o~ by the softmax denominators, apply w_fc +
                qc-chunk residual + mha_ln, ship bf16 rows to DRAM."""
                for jj in range(2):
                    j = 2 * t3 + jj
                    tpo = tpool.tile([P, 2, 81], f32, tag="tp", name="tpo")
                    for pair in range(2):
                        nc.tensor.transpose(
                            tpo[:, pair, :],
                            oT_sb[0:81, pair, t3, P * jj:P * (jj + 1)],
                            idn[0:81, 0:81])
                    rec = tmp.tile([P, 2, 2], f32, tag="rec")
                    nc.vector.reciprocal(rec[:, :, 0:1], tpo[:, :, DV:DV + 1])
                    nc.vector.reciprocal(rec[:, :, 1:2], tpo[:, :, D + DV:D + DV + 1])
                    for h in range(H):
                        pair, half = h // 2, h % 2
                        nc.vector.tensor_scalar_mul(
                            onr_bf[:, j, DV * h:DV * (h + 1)],
                            tpo[:, pair, D * half:D * half + DV],
                            rec[:, pair, half:half + 1])
                    tpn = tpool.tile([D, P], bf16, tag="tp", name="tpn")
                    nc.tensor.transpose(tpn[:, :], onr_bf[:, j, :], idnb[:, :])
                    if ceng is nc.scalar:
                        nc.scalar.activation(onT_bf[:, P * j:P * (j + 1)], tpn[:, :],
                                             AF.Copy, scale=1.0)
                    else:
                        nc.vector.tensor_copy(onT_bf[:, P * j:P * (j + 1)], tpn[:, :])
                    o2p = tpool.tile([P, D], f32, tag="tp", name="o2p")
                    nc.tensor.matmul(o2p[:, :], lhsT=onT_bf[:, P * j:P * (j + 1)],
                                     rhs=wfc_sb[:, :], start=True, stop=True)
                    nc.vector.tensor_add(o2r_sb[:, j, :], o2p[:, :], zr[:, j, :])
                    st2 = tmp.tile([P, 6], f32, tag="bnst2")
                    nc.vector.bn_stats(st2[:, :], o2r_sb[:, j, :])
                    nc.vector.bn_aggr(mv2[:, j, :], st2[:, :])
                    # per-jj LN finish so the first row's pl write issues early
                    ve2 = tmp.tile([P, 1], f32, tag="ve2", name=f"ve2{t3}{jj}")
                    nc.vector.tensor_scalar_add(ve2[:, :], mv2[:, j, 1:2], 1e-6)
                    rsqrt_newton(rst2[:, j:j + 1], ve2[:, :], [P, 1], f"ml{jj}")
                    nc.vector.scalar_tensor_tensor(
                        out=ol1_sb[:, j, :], in0=o2r_sb[:, j, :],
                        scalar=mv2[:, j, 0:1], in1=mw_sb[:, :],
                        op0=OP.subtract, op1=OP.mult)
                    nc.vector.tensor_scalar(
                        out=oln_bf[:, j, :], in0=ol1_sb[:, j, :],
                        scalar1=rst2[:, j:j + 1], scalar2=None, op0=OP.mult)
                    nc.gpsimd.tensor_add(oln_bf[:, j, :], oln_bf[:, j, :], mb_sb[:, :])
                    nc.sync.dma_start(pl_rows[:, j, :], oln_bf[:, j, :])

            def epilogue_half(n2, tpool, ceng=None):
                """Pool-gather + conv/relu/lin/residual + final LN for output
                node half n2 (needs oln rows j = 3*n2 .. 3*n2+2)."""
                nc.sync.dma_start(
                    pool_in[:, n2, :],
                    pl[:, :].rearrange("(n p s) d -> p n (s d)", n=2, p=P, s=3)[:, n2, :])
                xpool = tmp.tile([P, D], bf16, tag="xpool", name=f"xpool{n2}")
                with nc.allow_low_precision(reason="3-way pool mean in bf16"):
                    nc.vector.tensor_reduce(
                        xpool[:, :], pool_in[:, n2, :].rearrange("p (j s) -> p j s", s=3),
                        axis=mybir.AxisListType.X, op=OP.add)
                tpx = tpool.tile([D, P], bf16, tag="tp", name=f"tpx{n2}")
                nc.tensor.transpose(tpx[:, :], xpool[:, :], idnb[:, :])
                if ceng is nc.scalar:
                    nc.scalar.activation(xT_bf[:, P * n2:P * (n2 + 1)], tpx[:, :],
                                         AF.Copy, scale=1.0)
                else:
                    nc.vector.tensor_copy(xT_bf[:, P * n2:P * (n2 + 1)], tpx[:, :])
                x1ps = tpool.tile([DOUT, P], f32, tag="tp", name=f"x1ps{n2}")
                nc.tensor.matmul(x1ps[:, :], lhsT=convw_sb[:, :],
                                 rhs=xT_bf[:, P * n2:P * (n2 + 1)], start=True, stop=True)
                nc.scalar.activation(x1_bf[:, P * n2:P * (n2 + 1)], x1ps[:, :],
                                     AF.Relu, bias=convb_sb[:, :], scale=1.0)
                x2ps = tpool.tile([D, P], f32, tag="tp", name=f"x2ps{n2}")
                nc.tensor.matmul(x2ps[:, :], lhsT=linw_sb[:, :],
                                 rhs=x1_bf[:, P * n2:P * (n2 + 1)], start=True, stop=True)
                nc.vector.scalar_tensor_tensor(
                    out=x3T_sb[:, P * n2:P * (n2 + 1)], in0=x2ps[:, :],
                    scalar=linb_sb[:, :], in1=rest_sb[:, P * n2:P * (n2 + 1)],
                    op0=OP.add, op1=OP.add)
                tpf = tpool.tile([P, D], f32, tag="tp", name=f"tpf{n2}")
                nc.tensor.transpose(tpf[:, :], x3T_sb[:, P * n2:P * (n2 + 1)], idn[:D, :D])
                nc.vector.tensor_copy(xr_sb[:, n2, :], tpf[:, :])
                st3 = tmp.tile([P, 6], f32, tag="bnst3")
                nc.vector.bn_stats(st3[:, :], xr_sb[:, n2, :])
                nc.vector.bn_aggr(mv3[:, n2, :], st3[:, :])
                ve3 = tmp.tile([P, 1], f32, tag=f"ve3{n2}", name=f"ve3{n2}")
                nc.vector.tensor_scalar_add(ve3[:, :], mv3[:, n2, 1:2], 1e-5)
                rsqrt_newton(rst3[:, n2:n2 + 1], ve3[:, :], [P, 1], f"fl{n2}")
                nc.vector.scalar_tensor_tensor(
                    out=yout[:, n2, :], in0=xr_sb[:, n2, :],
                    scalar=mv3[:, n2, 0:1], in1=nw_sb[:, :],
                    op0=OP.subtract, op1=OP.mult)
                nc.vector.tensor_scalar(
                    out=yout[:, n2, :], in0=yout[:, n2, :],
                    scalar1=rst3[:, n2:n2 + 1], scalar2=None, op0=OP.mult)
                nc.gpsimd.tensor_add(yout[:, n2, :], yout[:, n2, :], nb_sb[:, :])
                nc.sync.dma_start(
                    out_d[:, :].rearrange("(t p) d -> p t d", p=P)[:, n2, :],
                    yout[:, n2, :])

            src_q = [xqb_sb, d_rows_f[("q", 1)], d_rows_f[("q", 2)]]
            src_v = [xvb_sb, d_rows_f[("v", 1)], d_rows_f[("v", 2)]]

            with (
                tc.tile_pool(name="tp", bufs=2, space="PSUM") as tp_pool,
                tc.tile_pool(name="dramp", bufs=1, space="DRAM") as dramp,
            ):
                pl = dramp.tile([CH, D], bf16)
                pl_rows = pl[:, :].rearrange("(t p) d -> p t d", p=P)
                # ===== step-0 q-side layernorm first: x tiles land ~1us in, so
                # DVE stats/applies and PE transposes run while acq streams =====
                ln_group(src_q, list(range(NT)), qc_rows, mv_q, rs_q, 1e-5, qcT_bf, tp_pool,
                         ceng=nc.scalar)
                # ===== chunk path: z = A_chunk x (feature-major), LN row-wise
                # via bf16 transpose -> bn_stats -> apply -> bf16 transpose =====
                with tc.tile_pool(name="chk", bufs=1, space="PSUM") as chk:
                    zps = chk.tile([D, CH], f32, tag="zps")
                    for t in range(NT):
                        nc.tensor.matmul(zps[:, 0:512], lhsT=xqb_sb[:, t, :],
                                         rhs=acq_sb[:, t, 0:512], start=(t == 0), stop=(t == NT - 1))
                        nc.tensor.matmul(zps[:, 512:CH], lhsT=xqb_sb[:, t, :],
                                         rhs=acq_sb[:, t, 512:CH], start=(t == 0), stop=(t == NT - 1))
                    nc.scalar.activation(zT_bf[:, :], zps[:, :], AF.Copy, scale=1.0)
                    for j0, cnt in ((0, 4), (4, 2)):
                        tq = tp_pool.tile([P, 4, D], bf16, tag="tp", name="ztq")
                        for m in range(cnt):
                            nc.tensor.transpose(tq[:, m, :], zT_bf[:, P * (j0 + m):P * (j0 + m + 1)],
                                                idnb[:D, :D])
                        nc.vector.tensor_copy(zr[:, j0:j0 + cnt, :], tq[:, 0:cnt, :])
                    mvc = tmp.tile([P, 6, 2], f32, tag="mvc")
                    for j in range(6):
                        stc = tmp.tile([P, 6], f32, tag="bnst")
                        nc.vector.bn_stats(stc[:, :], zr[:, j, :])
                        nc.vector.bn_aggr(mvc[:, j, :], stc[:, :])
                    vec = tmp.tile([P, 6], f32, tag="vec")
                    nc.vector.tensor_scalar_add(vec[:, :], mvc[:, :, 1], 1e-5)
                    rsc = tmp.tile([P, 6], f32, tag="rsc")
                    rsqrt_newton(rsc[:, :], vec[:, :], [P, 6], "chk")
                    for j in range(6):
                        nc.vector.tensor_scalar(
                            out=zr[:, j, :], in0=zr[:, j, :],
                            scalar1=mvc[:, j, 0:1], scalar2=rsc[:, j:j + 1],
                            op0=OP.subtract, op1=OP.mult)
                        nc.gpsimd.tensor_copy(zrb[:, j, :], zr[:, j, :])
                    for j0, cnt in ((0, 4), (4, 2)):
                        tq2 = tp_pool.tile([D, 4, P], bf16, tag="tp", name="ztq2")
                        for m in range(cnt):
                            nc.tensor.transpose(tq2[:, m, :], zrb[:, j0 + m, :], idnb[:, :])
                        nc.scalar.activation(
                            qcTc_bf[0:D, P * j0:P * (j0 + cnt)],
                            tq2[:, 0:cnt, :].rearrange("d m p -> d (m p)"),
                            AF.Copy, scale=1.0)
                    # Q~ per head (bf16)
                    for h in range(H):
                        qps = chk.tile([D, CH], f32, tag="sb", name="qps")
                        nc.tensor.matmul(qps[:, 0:512], lhsT=m_sb[:, D * h:D * (h + 1)],
                                         rhs=qcTc_bf[:, 0:512], start=True, stop=True)
                        nc.tensor.matmul(qps[:, 512:CH], lhsT=m_sb[:, D * h:D * (h + 1)],
                                         rhs=qcTc_bf[:, 512:CH], start=True, stop=True)
                        nc.scalar.activation(qT_bf[0:D, h, :], qps[:, :], AF.Copy, scale=1.0)

                # ===== step-0 v-side + V rows (feed av(kt0) ~2 iters in) =====
                ln_group(src_v, list(range(NT)), vc_rows, mv_v, rs_v, 1e-5, vcT_bf, tp_pool,
                         ceng=nc.scalar)
                v_rows(list(range(NT)), tp_pool, ceng=nc.scalar)

                # ===== diffusion task list (row-major accumulation; A^T is lhsT);
                # emitted interleaved between attention iterations =====
                def diffuse_tile(at_sb, lhs_src, dst_f, dst_b, i):
                    dps = tp_pool.tile([P, D], f32, tag="tp", name="dps")
                    for j in range(NT):
                        nc.tensor.matmul(
                            dps[:, :], lhsT=at_sb[:, j, P * i:P * (i + 1)],
                            rhs=lhs_src[:, j, :],
                            start=(j == 0), stop=(j == NT - 1))
                    nc.vector.tensor_copy(dst_f[:, i, :], dps[:, :])
                    if dst_b is not None:
                        nc.vector.tensor_copy(dst_b[:, i, :], dps[:, :])

                diff_tasks = []
                for at_sb_, lhs_, dstf_, dstb_ in (
                    (atq_sb, xqb_sb, d_rows_f[("q", 1)], dq1b),
                    (atv_sb, xvb_sb, d_rows_f[("v", 1)], dv1b),
                    (atq_sb, dq1b, d_rows_f[("q", 2)], None),
                    (atv_sb, dv1b, d_rows_f[("v", 2)], None),
                ):
                    for i_ in range(NT):
                        diff_tasks.append((at_sb_, lhs_, dstf_, dstb_, i_))
                diff_tasks = diff_tasks[::-1]  # pop from the end

                def emit_diff(n):
                    for _ in range(n):
                        if diff_tasks:
                            diffuse_tile(*diff_tasks.pop())

                with (
                    tc.tile_pool(name="psE", bufs=2, space="PSUM") as psE,
                    tc.tile_pool(name="psEa", bufs=1, space="PSUM") as psEa,
                    tc.tile_pool(name="expp", bufs=3) as expp,
                ):
                    # one third at a time: a PSUM bank must not host two
                    # accumulation groups split along the free dim; heads at
                    # partition bases 0/64 within a tile are fine.
                    def av_tiles(t3):
                        return [psEa.tile([D + DV1, QT3], f32, tag=f"avP{p}", name=f"av{t3}{p}")
                                for p in range(2)]

                    def flush(t3, avs):
                        for pair in range(2):
                            if t3 == 2:
                                nc.scalar.activation(oT_sb[0:D + DV1, pair, t3, :],
                                                     avs[pair][0:D + DV1, :],
                                                     AF.Copy, scale=1.0)
                            else:
                                nc.vector.tensor_copy(oT_sb[0:D + DV1, pair, t3, :],
                                                      avs[pair][0:D + DV1, :])

                    # software-pipelined attention: scores(kt) -> av(kt-1) ->
                    # exp(kt); the PE runs scores of kt+1 while ACT does exp(kt).
                    # Diffusion starts at kt 4 (atq/atv DMA lands ~14us);
                    # 4 tiles/iter over kt 4..7 finishes q1+v1 for grp 1's LN,
                    # then 2/iter over kt 8..15 finishes q2+v2 for grp 2.
                    avs = av_tiles(0)
                    av_fn = lambda h: avs[h // 2][D * (h % 2):D * (h % 2) + DV1, :]
                    pend = None
                    for grp in range(3):
                        if grp > 0:
                            kts = list(range(grp * NT, (grp + 1) * NT))
                            ln_group(src_q, kts, qc_rows, mv_q, rs_q, 1e-5, qcT_bf, tp_pool)
                            ln_group(src_v, kts, vc_rows, mv_v, rs_v, 1e-5, vcT_bf, tp_pool)
                            v_rows(kts, tp_pool)
                        for kt in range(grp * NT, (grp + 1) * NT):
                            sc = scores_iter(0, kt, psE)
                            if pend is not None:
                                av_iter(pend[0], pend[1], av_fn)
                            ex = exp_iter(sc, expp)
                            pend = (kt, ex)
                            if kt >= 4:
                                emit_diff(4 if kt < NT else 2)
                    av_iter(pend[0], pend[1], av_fn)
                    pend = None
                    emit_diff(32)
                    flush(0, avs)
                    # o_chain/epilogue work for finished thirds is deferred a
                    # few kt into the NEXT third so the in-order PE queue never
                    # stalls on flush copies / the pool-gather DMA.
                    for t3 in (1, 2):
                        avs = av_tiles(t3)
                        av_fn = lambda h, a=avs: a[h // 2][D * (h % 2):D * (h % 2) + DV1, :]
                        for kt in range(LT):
                            sc = scores_iter(t3, kt, psE)
                            if pend is not None:
                                av_iter(pend[0], pend[1], av_fn)
                            ex = exp_iter(sc, expp)
                            pend = (kt, ex)
                            if kt == 2:
                                o_chain(t3 - 1, tp_pool)
                            if t3 == 2 and kt == 6:
                                epilogue_half(0, tp_pool)
                        av_iter(pend[0], pend[1], av_fn)
                        pend = None
                        flush(t3, avs)

                # ================= tail (psE/psEa freed; tp_pool, pl alive) ==
                o_chain(2, tp_pool, ceng=nc.scalar)
                epilogue_half(1, tp_pool, ceng=nc.scalar)

    nc.finalize()
    return nc


def _prep_in_maps(inputs):
    import ml_dtypes
    bf = ml_dtypes.bfloat16

    q_x = np.asarray(inputs["q_x"], np.float32)
    v_x = np.asarray(inputs["v_x"], np.float32)
    q_adj = np.asarray(inputs["q_adj"], np.float32)
    v_adj = np.asarray(inputs["v_adj"], np.float32)
    w_qs = np.asarray(inputs["w_qs"], np.float32)
    w_ks = np.asarray(inputs["w_ks"], np.float32)
    w_vs = np.asarray(inputs["w_vs"], np.float32)
    w_fc = np.asarray(inputs["w_fc"], np.float32)
    mha_ln_w = np.asarray(inputs["mha_ln_w"], np.float32)
    mha_ln_b = np.asarray(inputs["mha_ln_b"], np.float32)
    conv_w = np.asarray(inputs["conv_w"], np.float32)
    conv_b = np.asarray(inputs["conv_b"], np.float32)
    lin_w = np.asarray(inputs["lin_w"], np.float32)
    lin_b = np.asarray(inputs["lin_b"], np.float32)
    norm_w = np.asarray(inputs["norm_w"], np.float32)
    norm_b = np.asarray(inputs["norm_b"], np.float32)

    # M_h = (Wq_h @ Wk_h^T) / sqrt(DK), stacked along columns
    m_all = np.zeros((D, H * D), np.float32)
    for h in range(H):
        m_all[:, D * h:D * (h + 1)] = (
            w_qs[:, DK * h:DK * (h + 1)] @ w_ks[:, DK * h:DK * (h + 1)].T
        ) / np.sqrt(DK)
    # augmented V projection: per head 16 value cols + a ones col (row 64)
    wv_aug = np.zeros((D + 1, H * DV1), np.float32)
    for h in range(H):
        wv_aug[:D, DV1 * h:DV1 * h + DV] = w_vs[:, DV * h:DV * (h + 1)]
        wv_aug[D, DV1 * h + DV] = 1.0
    conv_w3 = conv_w / 3.0

    shared = dict(
        m_bf=m_all.astype(bf),
        wv_aug=wv_aug.astype(bf),
        wfc=w_fc.astype(bf), mha_w=mha_ln_w, mha_b=mha_ln_b,
        conv_w3=conv_w3.astype(bf), conv_b=conv_b,
        lin_w=lin_w.astype(bf), lin_b=lin_b, norm_w=norm_w, norm_b=norm_b,
    )

    per_batch = []
    for b in range(B):
        A, Av = q_adj[b], v_adj[b]
        A2 = A @ A
        G = np.concatenate([np.eye(N, dtype=np.float32), A, A2], axis=0)  # [3N, N]
        per_batch.append(dict(
            xqb=q_x[b].astype(bf),
            xvb=v_x[b].astype(bf),
            atq=np.ascontiguousarray(A.T).astype(bf),
            atv=np.ascontiguousarray(Av.T).astype(bf),
            G=G,
        ))

    in_maps = []
    for c in range(8):
        b, g = c // 4, c % 4
        pb = per_batch[b]
        acq = np.ascontiguousarray(pb["G"][CH * g:CH * (g + 1)].T).astype(bf)  # [N, CH]
        rest = np.ascontiguousarray(q_x[b, CN * g:CN * (g + 1)].T)             # [D, CN]
        m = dict(shared)
        m.update(xqb=pb["xqb"], xvb=pb["xvb"],
                 atq=pb["atq"], atv=pb["atv"], acq=acq, rest=rest)
        in_maps.append(m)
    return in_maps


def _run(inputs, trace=False, **kw):
    from concourse.bass_utils import run_bass_kernel_spmd

    if "nc" not in _CACHE:
        _CACHE["nc"] = _build_nc()
    nc = _CACHE["nc"]
    in_maps = _prep_in_maps(inputs)
    res = run_bass_kernel_spmd(nc, in_maps, core_ids=list(range(8)), trace=trace, **kw)
    out = np.empty((B, N, D), np.float32)
    for c in range(8):
        b, g = c // 4, c % 4
        out[b, CN * g:CN * (g + 1)] = res.results[c]["out"]
    return out, res


def kernel(**inputs) -> np.ndarray:
    out, _ = _run(inputs, trace=False)
    return out


# revision 16
# speedup vs baseline: 1.0252x; 1.0252x over previous
"""Trainium2 Bass kernel for nn_DiffAtten (diffusion GNN + multi-head attention).

Model (per batch b): qc = LN([x; Ax; A^2x]) (L=3072 rows), vc likewise with v-graph;
MHA over L with H=4 heads of dim 16; o = attn-out @ w_fc + qc; LN; pool triples of
rows; conv+relu+linear+residual; final LN.  Output [2, 1024, 64] f32.

Sharding: 8 cores = 2 batches x 4 groups.  Core (b, g) computes attention for the
L-contiguous query chunk [768g, 768(g+1)) (which exactly covers output nodes
[256g, 256(g+1)) after the triple-pooling), using the full K/V side (3072 keys)
computed on-core from the full adjacency of batch b.  No collectives; all per-core
specialisation is carried by host-side input slicing:
  - atq/atv: A^T (pre-transposed, bf16); stationary matmul operand for diffusion.
  - acq: transposed chunk-operator rows of [I; A; A^2] for the core's L-chunk.
  - m_bf: per-head M_h = (Wq_h Wk_h^T)/sqrt(dk), so scores = qc M_h qc_chunk^T.
  - wv_aug: V projection augmented with a ones row/column per head; the
    attention@V matmul then emits softmax denominators for free.

Perf notes (v2): everything the PE streams is bf16 (1 cycle/col vs 4 for f32);
score matmuls are packed 2-heads-per-matmul (512 free columns); the attention
loop is software-pipelined (scores of kt+1 issue before AV of kt so the in-order
PE never waits on the exp); LN applies / PSUM drains run on the idle Pool engine;
rsqrt is 2 Newton steps on DVE; inputs stream in per-tile so compute starts ~2us
in; the triple-pool epilogue is split so only the n=1 half waits on the last
attention third.
"""

import numpy as np

B, N, D = 2, 1024, 64
H, DK, DV = 4, 16, 16
DOUT = 128
STEPS = 3
L = STEPS * N          # 3072
P = 128
NT = N // P            # 8 node tiles
LT = L // P            # 24 L tiles
CH = L // 4            # 768 q-chunk per core
CN = N // 4            # 256 output nodes per core
QT3 = CH // 3          # 256 q columns per inner third
DV1 = DV + 1           # 17
RSQRT_MAGIC = 0x5F3759DF

_CACHE = {}


def _bcast_ap(bass_mod, ap, parts):
    """[F] dram AP -> [parts, F] broadcast AP (partition step 0)."""
    return bass_mod.AP(tensor=ap.tensor, offset=ap.offset, ap=[[0, parts]] + list(ap.ap))


def _build_nc():
    import concourse.bass as bass
    import concourse.bacc as bacc
    import concourse.tile as tile
    from concourse import mybir, masks

    f32 = mybir.dt.float32
    i32 = mybir.dt.int32
    bf16 = mybir.dt.bfloat16
    AF = mybir.ActivationFunctionType
    OP = mybir.AluOpType

    nc = bacc.Bacc(None, target_bir_lowering=False)

    # ---- kernel I/O (per-core slices supplied by the host) ----
    xqb = nc.dram_tensor("xqb", [N, D], bf16, kind="ExternalInput")
    xvb = nc.dram_tensor("xvb", [N, D], bf16, kind="ExternalInput")
    atq = nc.dram_tensor("atq", [N, N], bf16, kind="ExternalInput")
    atv = nc.dram_tensor("atv", [N, N], bf16, kind="ExternalInput")
    acq = nc.dram_tensor("acq", [N, CH], bf16, kind="ExternalInput")
    m_bf = nc.dram_tensor("m_bf", [D, H * D], bf16, kind="ExternalInput")
    wv_aug = nc.dram_tensor("wv_aug", [D + 1, H * DV1], bf16, kind="ExternalInput")
    wfc = nc.dram_tensor("wfc", [D, D], bf16, kind="ExternalInput")
    mha_w = nc.dram_tensor("mha_w", [D], f32, kind="ExternalInput")
    mha_b = nc.dram_tensor("mha_b", [D], f32, kind="ExternalInput")
    conv_w3 = nc.dram_tensor("conv_w3", [D, DOUT], bf16, kind="ExternalInput")
    conv_b = nc.dram_tensor("conv_b", [DOUT], f32, kind="ExternalInput")
    lin_w = nc.dram_tensor("lin_w", [DOUT, D], bf16, kind="ExternalInput")
    lin_b = nc.dram_tensor("lin_b", [D], f32, kind="ExternalInput")
    norm_w = nc.dram_tensor("norm_w", [D], f32, kind="ExternalInput")
    norm_b = nc.dram_tensor("norm_b", [D], f32, kind="ExternalInput")
    rest = nc.dram_tensor("rest", [D, CN], f32, kind="ExternalInput")
    out_d = nc.dram_tensor("out", [CN, D], f32, kind="ExternalOutput")

    with tile.TileContext(nc) as tc:
        with (
            tc.tile_pool(name="consts", bufs=1) as consts,
            tc.tile_pool(name="big", bufs=1) as big,
            tc.tile_pool(name="tmp", bufs=4) as tmp,
            tc.tile_pool(name="ntmp", bufs=2) as ntmp,
        ):
            # ---------------- constants ----------------
            idn = consts.tile([P, P], f32)
            masks.make_identity(nc, idn[:, :])
            idnb = consts.tile([P, P], bf16)
            masks.make_identity(nc, idnb[:, :])

            # ---- input DMAs, in consumption order.  The chunk path (xqb, acq)
            # gates everything; acq is split so z-matmuls start ~2us in.
            xqb_sb = big.tile([P, NT, D], bf16)
            nc.sync.dma_start(xqb_sb[:, :, :], xqb[:, :].rearrange("(t p) d -> p t d", p=P))
            xvb_sb = big.tile([P, NT, D], bf16)
            nc.sync.dma_start(xvb_sb[:, :, :], xvb[:, :].rearrange("(t p) d -> p t d", p=P))
            m_sb = consts.tile([P, H * D], bf16)   # padded to 128 rows (zeros below)
            nc.scalar.dma_start(m_sb[0:D, :], m_bf[:, :])
            wva_sb = consts.tile([D + 1, H * DV1], bf16)
            nc.scalar.dma_start(wva_sb[:, :], wv_aug[:, :])
            wfc_sb = consts.tile([D, D], bf16)
            nc.scalar.dma_start(wfc_sb[:, :], wfc[:, :])
            convw_sb = consts.tile([D, DOUT], bf16)
            nc.scalar.dma_start(convw_sb[:, :], conv_w3[:, :])
            convb_sb = consts.tile([DOUT, 1], f32)
            nc.scalar.dma_start(convb_sb[:, :], conv_b[:].unsqueeze(1))
            linw_sb = consts.tile([DOUT, D], bf16)
            nc.scalar.dma_start(linw_sb[:, :], lin_w[:, :])
            linb_sb = consts.tile([D, 1], f32)
            nc.scalar.dma_start(linb_sb[:, :], lin_b[:].unsqueeze(1))
            rest_sb = consts.tile([D, CN], f32)
            nc.scalar.dma_start(rest_sb[:, :], rest[:, :])
            mw_sb = consts.tile([P, D], f32)
            nc.scalar.dma_start(mw_sb[:, :], _bcast_ap(bass, mha_w[:], P))
            mb_sb = consts.tile([P, D], f32)
            nc.scalar.dma_start(mb_sb[:, :], _bcast_ap(bass, mha_b[:], P))
            nw_sb = consts.tile([P, D], f32)
            nc.scalar.dma_start(nw_sb[:, :], _bcast_ap(bass, norm_w[:], P))
            nb_sb = consts.tile([P, D], f32)
            nc.scalar.dma_start(nb_sb[:, :], _bcast_ap(bass, norm_b[:], P))
            acq_sb = big.tile([P, NT, CH], bf16)  # chunk operator^T (own DMA queue)
            for hf in range(2):
                t0 = hf * (NT // 2)
                nc.gpsimd.dma_start(
                    acq_sb[:, t0:t0 + NT // 2, :],
                    acq[t0 * P:(t0 + NT // 2) * P, :].rearrange("(t p) c -> p t c", p=P))
            # bulk adjacency tiles; DMA dispatch deferred until the prologue
            # compute is done — the 4MB stream starves DVE/PE SBUF access.
            atq_sb = big.tile([P, NT, N], bf16)   # A_q^T, row jt = t*128+p
            atv_sb = big.tile([P, NT, N], bf16)

            # persistent intermediates
            d_rows_f = {}   # (side, step) -> [128, 8, 64] f32 rows of A^s x
            for side in ("q", "v"):
                for step in (1, 2):
                    d_rows_f[(side, step)] = big.tile(
                        [P, NT, D], f32, tag=f"d{side}{step}f", name=f"d{side}{step}f")
            dq1b = big.tile([P, NT, D], bf16)
            dv1b = big.tile([P, NT, D], bf16)

            qc_rows = big.tile([P, LT, D], bf16)   # LN'd rows (bf16, feed transposes)
            vc_rows = big.tile([P, LT, D], bf16)
            qcT_bf = big.tile([P, L], bf16)    # rows D: zero (contract-128 pad)
            vcT_bf = big.tile([D + 1, L], bf16)
            vrows_bf = big.tile([P, LT, H * DV1], bf16)
            qT_bf = big.tile([P, H, CH], bf16)
            qcTc_bf = big.tile([P, CH], bf16)       # qc chunk^T (bf16, padded)
            mv_q = big.tile([P, LT, 2], f32)
            mv_v = big.tile([P, LT, 2], f32)
            rs_q = big.tile([P, LT], f32)
            rs_v = big.tile([P, LT], f32)
            # o~^T: dim1 = head pair; heads of a pair at partition bases 0 / 64
            oT_sb = big.tile([P, 2, 3, QT3], f32)
            onr_bf = big.tile([P, 6, D], bf16)      # normalized attn out rows
            onT_bf = big.tile([D, CH], bf16)
            o2r_sb = big.tile([P, 6, D], f32)       # (o@wfc + qc) rows
            ol1_sb = big.tile([P, 6, D], f32)       # (o2 - mu) * w scratch
            oln_bf = big.tile([P, 6, D], bf16)      # after mha_ln (bf16 rows)
            zr = big.tile([P, 6, D], f32)           # chunk qc rows (f32, residual)
            zrb = big.tile([P, 6, D], bf16)
            zT_bf = big.tile([D, CH], bf16)
            mv2 = big.tile([P, 6, 2], f32)
            rst2 = big.tile([P, 6], f32)
            pool_in = big.tile([P, 2, 3 * D], bf16)
            xT_bf = big.tile([D, CN], bf16)
            x1_bf = big.tile([DOUT, CN], bf16)
            x3T_sb = big.tile([D, CN], f32)
            xr_sb = big.tile([P, 2, D], f32)
            yout = big.tile([P, 2, D], f32)
            mv3 = big.tile([P, 2, 2], f32)
            rst3 = big.tile([P, 2], f32)

            nc.gpsimd.memset(vcT_bf[D:D + 1, :], 1.0)   # ones row for V denominators
            # zero pads: the PE streams contract-64 operands at half rate, so
            # scores/qps matmuls carry 64 zero rows to keep contract=128.
            nc.gpsimd.memset(qcT_bf[D:P, :], 0.0)
            nc.vector.memset(qT_bf[D:P, :, :], 0.0)
            nc.gpsimd.memset(qcTc_bf[D:P, :], 0.0)
            nc.gpsimd.memset(m_sb[D:P, :], 0.0)

            def rsqrt_newton(dst, src, shape, tag, iters=1):
                """dst = 1/sqrt(src) elementwise via fast-inverse-sqrt + Newton.
                src must be > 0. shape = [parts, free]. All on DVE.
                1 iteration: rel err ~1.7e-3; LN row-scale errors largely
                cancel (softmax is scale-covariant per row, LN renormalizes)."""
                hv = ntmp.tile(shape, f32, tag=tag + "h", name=tag + "h")
                nc.vector.tensor_scalar_mul(hv[:, :], src, 0.5)
                y = dst
                nc.vector.tensor_scalar(
                    out=y.bitcast(i32), in0=src.bitcast(i32),
                    scalar1=1, scalar2=None, op0=OP.logical_shift_right)
                nc.vector.tensor_scalar(
                    out=y.bitcast(i32), in0=y.bitcast(i32),
                    scalar1=-1, scalar2=None, op0=OP.bitwise_xor)
                nc.vector.tensor_scalar(
                    out=y.bitcast(i32), in0=y.bitcast(i32),
                    scalar1=RSQRT_MAGIC + 1, scalar2=None, op0=OP.add)
                t = ntmp.tile(shape, f32, tag=tag + "t", name=tag + "t")
                for _ in range(iters):
                    nc.vector.tensor_mul(t[:, :], y, y)
                    nc.vector.tensor_tensor(out=t[:, :], in0=t[:, :], in1=hv[:, :], op=OP.mult)
                    nc.vector.tensor_scalar(
                        out=t[:, :], in0=t[:, :], scalar1=-1.0, scalar2=1.5,
                        op0=OP.mult, op1=OP.add)
                    nc.vector.tensor_mul(y, y, t[:, :])

            def ln_group(srcs, kts, rows, mv, rs, eps, tdst, tpool, ceng=None):
                """Stats (DVE) + apply (Pool, bf16 out) + bf16 transpose (PE) for
                row tiles kts; write bf16 rows and transposed columns of tdst."""
                for i in kts:
                    st = tmp.tile([P, 6], f32, tag="bnst")
                    nc.vector.bn_stats(st[:, :], srcs[i // NT][:, i % NT, :])
                    nc.vector.bn_aggr(mv[:, i, :], st[:, :])
                i0, n = kts[0], len(kts)
                ve = tmp.tile([P, LT], f32, tag="ve")
                nc.vector.tensor_scalar_add(ve[:, i0:i0 + n], mv[:, i0:i0 + n, 1], eps)
                rsqrt_newton(rs[:, i0:i0 + n], ve[:, i0:i0 + n], [P, n], "lng")
                for i in kts:
                    nc.vector.tensor_scalar(
                        out=rows[:, i, :], in0=srcs[i // NT][:, i % NT, :],
                        scalar1=mv[:, i, 0:1], scalar2=rs[:, i:i + 1],
                        op0=OP.subtract, op1=OP.mult)
                for q0 in range(kts[0], kts[0] + len(kts), 4):
                    tpb = tpool.tile([D, 4, P], bf16, tag="tp")
                    for m in range(4):
                        nc.tensor.transpose(tpb[:, m, :], rows[:, q0 + m, :], idnb[:, :])
                    ap_dst = tdst[:D, P * q0:P * (q0 + 4)]
                    ap_src = tpb[:, :, :].rearrange("d m p -> d (m p)")
                    if ceng is nc.scalar:
                        nc.scalar.activation(ap_dst, ap_src, AF.Copy, scale=1.0)
                    else:
                        nc.vector.tensor_copy(ap_dst, ap_src)

            def v_rows(kts, tpool, ceng=None):
                for q0 in range(kts[0], kts[0] + len(kts), 4):
                    vps = tpool.tile([P, 4, H * DV1], f32, tag="tp")
                    for m in range(4):
                        nc.tensor.matmul(vps[:, m, :],
                                         lhsT=vcT_bf[:, P * (q0 + m):P * (q0 + m + 1)],
                                         rhs=wva_sb[:, :], start=True, stop=True)
                    if ceng is nc.scalar:
                        nc.scalar.activation(vrows_bf[:, q0:q0 + 4, :], vps[:, :, :],
                                             AF.Copy, scale=1.0)
                    else:
                        nc.vector.tensor_copy(vrows_bf[:, q0:q0 + 4, :], vps[:, :, :])

            def scores_iter(t3, kt, scp):
                sc = scp.tile([P, H, QT3], f32, tag="sc")
                for hp in range(2):
                    nc.tensor.matmul(
                        sc[:, 2 * hp:2 * hp + 2, :],
                        lhsT=qcT_bf[:, P * kt:P * (kt + 1)],
                        rhs=qT_bf[:, 2 * hp:2 * hp + 2, QT3 * t3:QT3 * (t3 + 1)],
                        start=True, stop=True)
                return sc

            def exp_iter(sc, expp):
                ex = expp.tile([P, H, QT3], bf16, tag="ex")
                nc.scalar.activation(ex[:, :, :], sc[:, :, :], AF.Exp, scale=1.0)
                return ex

            def av_iter(kt, ex, av_fn):
                for h in range(H):
                    nc.tensor.matmul(av_fn(h),
                                     lhsT=vrows_bf[:, kt, DV1 * h:DV1 * (h + 1)],
                                     rhs=ex[:, h, :],
                                     start=(kt == 0), stop=(kt == LT - 1),
                                     skip_group_check=True)

            def o_chain(t3, tpool, ceng=None):
                """Normalize o~ by the softmax denominators, apply w_fc +
                qc-chunk residual + mha_ln, ship bf16 rows to DRAM."""
                for jj in range(2):
                    j = 2 * t3 + jj
                    tpo = tpool.tile([P, 2, 81], f32, tag="tp", name="tpo")
                    for pair in range(2):
                        nc.tensor.transpose(
                            tpo[:, pair, :],
                            oT_sb[0:81, pair, t3, P * jj:P * (jj + 1)],
                            idn[0:81, 0:81])
                    rec = tmp.tile([P, 2, 2], f32, tag="rec")
                    nc.vector.reciprocal(rec[:, :, 0:1], tpo[:, :, DV:DV + 1])
                    nc.vector.reciprocal(rec[:, :, 1:2], tpo[:, :, D + DV:D + DV + 1])
                    for h in range(H):
                        pair, half = h // 2, h % 2
                        nc.vector.tensor_scalar_mul(
                            onr_bf[:, j, DV * h:DV * (h + 1)],
                            tpo[:, pair, D * half:D * half + DV],
                            rec[:, pair, half:half + 1])
                    tpn = tpool.tile([D, P], bf16, tag="tp", name="tpn")
                    nc.tensor.transpose(tpn[:, :], onr_bf[:, j, :], idnb[:, :])
                    if ceng is nc.scalar:
                        nc.scalar.activation(onT_bf[:, P * j:P * (j + 1)], tpn[:, :],
                                             AF.Copy, scale=1.0)
                    else:
                        nc.vector.tensor_copy(onT_bf[:, P * j:P * (j + 1)], tpn[:, :])
                    o2p = tpool.tile([P, D], f32, tag="tp", name="o2p")
                    nc.tensor.matmul(o2p[:, :], lhsT=onT_bf[:, P * j:P * (j + 1)],
                                     rhs=wfc_sb[:, :], start=True, stop=True)
                    nc.vector.tensor_add(o2r_sb[:, j, :], o2p[:, :], zr[:, j, :])
                    st2 = tmp.tile([P, 6], f32, tag="bnst2")
                    nc.vector.bn_stats(st2[:, :], o2r_sb[:, j, :])
                    nc.vector.bn_aggr(mv2[:, j, :], st2[:, :])
                    # per-jj LN finish so the first row's pl write issues early
                    ve2 = tmp.tile([P, 1], f32, tag="ve2", name=f"ve2{t3}{jj}")
                    nc.vector.tensor_scalar_add(ve2[:, :], mv2[:, j, 1:2], 1e-6)
                    rsqrt_newton(rst2[:, j:j + 1], ve2[:, :], [P, 1], f"ml{jj}")
                    nc.vector.scalar_tensor_tensor(
                        out=ol1_sb[:, j, :], in0=o2r_sb[:, j, :],
                        scalar=mv2[:, j, 0:1], in1=mw_sb[:, :],
                        op0=OP.subtract, op1=OP.mult)
                    nc.vector.tensor_scalar(
                        out=oln_bf[:, j, :], in0=ol1_sb[:, j, :],
                        scalar1=rst2[:, j:j + 1], scalar2=None, op0=OP.mult)
                    nc.gpsimd.tensor_add(oln_bf[:, j, :], oln_bf[:, j, :], mb_sb[:, :])
                    nc.sync.dma_start(pl_rows[:, j, :], oln_bf[:, j, :])

            def epilogue_half(n2, tpool, ceng=None):
                """Pool-gather + conv/relu/lin/residual + final LN for output
                node half n2 (needs oln rows j = 3*n2 .. 3*n2+2)."""
                nc.sync.dma_start(
                    pool_in[:, n2, :],
                    pl[:, :].rearrange("(n p s) d -> p n (s d)", n=2, p=P, s=3)[:, n2, :])
                xpool = tmp.tile([P, D], bf16, tag="xpool", name=f"xpool{n2}")
                with nc.allow_low_precision(reason="3-way pool mean in bf16"):
                    nc.vector.tensor_reduce(
                        xpool[:, :], pool_in[:, n2, :].rearrange("p (j s) -> p j s", s=3),
                        axis=mybir.AxisListType.X, op=OP.add)
                tpx = tpool.tile([D, P], bf16, tag="tp", name=f"tpx{n2}")
                nc.tensor.transpose(tpx[:, :], xpool[:, :], idnb[:, :])
                if ceng is nc.scalar:
                    nc.scalar.activation(xT_bf[:, P * n2:P * (n2 + 1)], tpx[:, :],
                                         AF.Copy, scale=1.0)
                else:
                    nc.vector.tensor_copy(xT_bf[:, P * n2:P * (n2 + 1)], tpx[:, :])
                x1ps = tpool.tile([DOUT, P], f32, tag="tp", name=f"x1ps{n2}")
                nc.tensor.matmul(x1ps[:, :], lhsT=convw_sb[:, :],
                                 rhs=xT_bf[:, P * n2:P * (n2 + 1)], start=True, stop=True)
                nc.scalar.activation(x1_bf[:, P * n2:P * (n2 + 1)], x1ps[:, :],
                                     AF.Relu, bias=convb_sb[:, :], scale=1.0)
                x2ps = tpool.tile([D, P], f32, tag="tp", name=f"x2ps{n2}")
                nc.tensor.matmul(x2ps[:, :], lhsT=linw_sb[:, :],
                                 rhs=x1_bf[:, P * n2:P * (n2 + 1)], start=True, stop=True)
                nc.vector.scalar_tensor_tensor(
                    out=x3T_sb[:, P * n2:P * (n2 + 1)], in0=x2ps[:, :],
                    scalar=linb_sb[:, :], in1=rest_sb[:, P * n2:P * (n2 + 1)],
                    op0=OP.add, op1=OP.add)
                tpf = tpool.tile([P, D], f32, tag="tp", name=f"tpf{n2}")
                nc.tensor.transpose(tpf[:, :], x3T_sb[:, P * n2:P * (n2 + 1)], idn[:D, :D])
                nc.vector.tensor_copy(xr_sb[:, n2, :], tpf[:, :])
                st3 = tmp.tile([P, 6], f32, tag="bnst3")
                nc.vector.bn_stats(st3[:, :], xr_sb[:, n2, :])
                nc.vector.bn_aggr(mv3[:, n2, :], st3[:, :])
                ve3 = tmp.tile([P, 1], f32, tag=f"ve3{n2}", name=f"ve3{n2}")
                nc.vector.tensor_scalar_add(ve3[:, :], mv3[:, n2, 1:2], 1e-5)
                rsqrt_newton(rst3[:, n2:n2 + 1], ve3[:, :], [P, 1], f"fl{n2}")
                nc.vector.scalar_tensor_tensor(
                    out=yout[:, n2, :], in0=xr_sb[:, n2, :],
                    scalar=mv3[:, n2, 0:1], in1=nw_sb[:, :],
                    op0=OP.subtract, op1=OP.mult)
                nc.vector.tensor_scalar(
                    out=yout[:, n2, :], in0=yout[:, n2, :],
                    scalar1=rst3[:, n2:n2 + 1], scalar2=None, op0=OP.mult)
                nc.gpsimd.tensor_add(yout[:, n2, :], yout[:, n2, :], nb_sb[:, :])
                nc.sync.dma_start(
                    out_d[:, :].rearrange("(t p) d -> p t d", p=P)[:, n2, :],
                    yout[:, n2, :])

            src_q = [xqb_sb, d_rows_f[("q", 1)], d_rows_f[("q", 2)]]
            src_v = [xvb_sb, d_rows_f[("v", 1)], d_rows_f[("v", 2)]]

            with (
                tc.tile_pool(name="tp", bufs=2, space="PSUM") as tp_pool,
                tc.tile_pool(name="dramp", bufs=1, space="DRAM") as dramp,
            ):
                pl = dramp.tile([CH, D], bf16)
                pl_rows = pl[:, :].rearrange("(t p) d -> p t d", p=P)
                # ===== step-0 q-side layernorm first: x tiles land ~1us in, so
                # DVE stats/applies and PE transposes run while acq streams =====
                ln_group(src_q, list(range(NT)), qc_rows, mv_q, rs_q, 1e-5, qcT_bf, tp_pool,
                         ceng=nc.scalar)
                # ===== chunk path: z = A_chunk x (feature-major), LN row-wise
                # via bf16 transpose -> bn_stats -> apply -> bf16 transpose =====
                with tc.tile_pool(name="chk", bufs=1, space="PSUM") as chk:
                    zps = chk.tile([D, CH], f32, tag="zps")
                    for t in range(NT):
                        nc.tensor.matmul(zps[:, 0:512], lhsT=xqb_sb[:, t, :],
                                         rhs=acq_sb[:, t, 0:512], start=(t == 0), stop=(t == NT - 1))
                        nc.tensor.matmul(zps[:, 512:CH], lhsT=xqb_sb[:, t, :],
                                         rhs=acq_sb[:, t, 512:CH], start=(t == 0), stop=(t == NT - 1))
                    nc.scalar.activation(zT_bf[:, :], zps[:, :], AF.Copy, scale=1.0)
                    for j0, cnt in ((0, 4), (4, 2)):
                        tq = tp_pool.tile([P, 4, D], bf16, tag="tp", name="ztq")
                        for m in range(cnt):
                            nc.tensor.transpose(tq[:, m, :], zT_bf[:, P * (j0 + m):P * (j0 + m + 1)],
                                                idnb[:D, :D])
                        nc.vector.tensor_copy(zr[:, j0:j0 + cnt, :], tq[:, 0:cnt, :])
                    mvc = tmp.tile([P, 6, 2], f32, tag="mvc")
                    for j in range(6):
                        stc = tmp.tile([P, 6], f32, tag="bnst")
                        nc.vector.bn_stats(stc[:, :], zr[:, j, :])
                        nc.vector.bn_aggr(mvc[:, j, :], stc[:, :])
                    vec = tmp.tile([P, 6], f32, tag="vec")
                    nc.vector.tensor_scalar_add(vec[:, :], mvc[:, :, 1], 1e-5)
                    rsc = tmp.tile([P, 6], f32, tag="rsc")
                    rsqrt_newton(rsc[:, :], vec[:, :], [P, 6], "chk")
                    for j in range(6):
                        nc.vector.tensor_scalar(
                            out=zr[:, j, :], in0=zr[:, j, :],
                            scalar1=mvc[:, j, 0:1], scalar2=rsc[:, j:j + 1],
                            op0=OP.subtract, op1=OP.mult)
                        nc.gpsimd.tensor_copy(zrb[:, j, :], zr[:, j, :])
                    for j0, cnt in ((0, 4), (4, 2)):
                        tq2 = tp_pool.tile([D, 4, P], bf16, tag="tp", name="ztq2")
                        for m in range(cnt):
                            nc.tensor.transpose(tq2[:, m, :], zrb[:, j0 + m, :], idnb[:, :])
                        nc.scalar.activation(
                            qcTc_bf[0:D, P * j0:P * (j0 + cnt)],
                            tq2[:, 0:cnt, :].rearrange("d m p -> d (m p)"),
                            AF.Copy, scale=1.0)
                    # Q~ per head (bf16)
                    for h in range(H):
                        qps = chk.tile([D, CH], f32, tag="sb", name="qps")
                        nc.tensor.matmul(qps[:, 0:512], lhsT=m_sb[:, D * h:D * (h + 1)],
                                         rhs=qcTc_bf[:, 0:512], start=True, stop=True)
                        nc.tensor.matmul(qps[:, 512:CH], lhsT=m_sb[:, D * h:D * (h + 1)],
                                         rhs=qcTc_bf[:, 512:CH], start=True, stop=True)
                        nc.scalar.activation(qT_bf[0:D, h, :], qps[:, :], AF.Copy, scale=1.0)

                nc.gpsimd.dma_start(atq_sb[:, :, :],
                                    atq[:, :].rearrange("(t p) i -> p t i", p=P))
                nc.gpsimd.dma_start(atv_sb[:, :, :],
                                    atv[:, :].rearrange("(t p) i -> p t i", p=P))
                # ===== step-0 v-side + V rows (feed av(kt0) ~2 iters in) =====
                ln_group(src_v, list(range(NT)), vc_rows, mv_v, rs_v, 1e-5, vcT_bf, tp_pool,
                         ceng=nc.scalar)
                v_rows(list(range(NT)), tp_pool, ceng=nc.scalar)

                # ===== diffusion task list (row-major accumulation; A^T is lhsT);
                # emitted interleaved between attention iterations =====
                def diffuse_tile(at_sb, lhs_src, dst_f, dst_b, i):
                    dps = tp_pool.tile([P, D], f32, tag="tp", name="dps")
                    for j in range(NT):
                        nc.tensor.matmul(
                            dps[:, :], lhsT=at_sb[:, j, P * i:P * (i + 1)],
                            rhs=lhs_src[:, j, :],
                            start=(j == 0), stop=(j == NT - 1))
                    nc.vector.tensor_copy(dst_f[:, i, :], dps[:, :])
                    if dst_b is not None:
                        nc.vector.tensor_copy(dst_b[:, i, :], dps[:, :])

                diff_tasks = []
                for at_sb_, lhs_, dstf_, dstb_ in (
                    (atq_sb, xqb_sb, d_rows_f[("q", 1)], dq1b),
                    (atv_sb, xvb_sb, d_rows_f[("v", 1)], dv1b),
                    (atq_sb, dq1b, d_rows_f[("q", 2)], None),
                    (atv_sb, dv1b, d_rows_f[("v", 2)], None),
                ):
                    for i_ in range(NT):
                        diff_tasks.append((at_sb_, lhs_, dstf_, dstb_, i_))
                diff_tasks = diff_tasks[::-1]  # pop from the end

                def emit_diff(n):
                    for _ in range(n):
                        if diff_tasks:
                            diffuse_tile(*diff_tasks.pop())

                with (
                    tc.tile_pool(name="psE", bufs=2, space="PSUM") as psE,
                    tc.tile_pool(name="psEa", bufs=1, space="PSUM") as psEa,
                    tc.tile_pool(name="expp", bufs=3) as expp,
                ):
                    # one third at a time: a PSUM bank must not host two
                    # accumulation groups split along the free dim; heads at
                    # partition bases 0/64 within a tile are fine.
                    def av_tiles(t3):
                        return [psEa.tile([D + DV1, QT3], f32, tag=f"avP{p}", name=f"av{t3}{p}")
                                for p in range(2)]

                    def flush(t3, avs):
                        for pair in range(2):
                            if t3 == 2:
                                nc.scalar.activation(oT_sb[0:D + DV1, pair, t3, :],
                                                     avs[pair][0:D + DV1, :],
                                                     AF.Copy, scale=1.0)
                            else:
                                nc.vector.tensor_copy(oT_sb[0:D + DV1, pair, t3, :],
                                                      avs[pair][0:D + DV1, :])

                    # software-pipelined attention: scores(kt) -> av(kt-1) ->
                    # exp(kt); the PE runs scores of kt+1 while ACT does exp(kt).
                    # Diffusion starts at kt 4 (atq/atv DMA lands ~14us);
                    # 4 tiles/iter over kt 4..7 finishes q1+v1 for grp 1's LN,
                    # then 2/iter over kt 8..15 finishes q2+v2 for grp 2.
                    avs = av_tiles(0)
                    av_fn = lambda h: avs[h // 2][D * (h % 2):D * (h % 2) + DV1, :]
                    pend = None
                    for grp in range(3):
                        if grp > 0:
                            kts = list(range(grp * NT, (grp + 1) * NT))
                            ln_group(src_q, kts, qc_rows, mv_q, rs_q, 1e-5, qcT_bf, tp_pool)
                            ln_group(src_v, kts, vc_rows, mv_v, rs_v, 1e-5, vcT_bf, tp_pool)
                            v_rows(kts, tp_pool)
                        for kt in range(grp * NT, (grp + 1) * NT):
                            sc = scores_iter(0, kt, psE)
                            if pend is not None:
                                av_iter(pend[0], pend[1], av_fn)
                            ex = exp_iter(sc, expp)
                            pend = (kt, ex)
                            if kt >= 4:
                                emit_diff(4 if kt < NT else 2)
                    av_iter(pend[0], pend[1], av_fn)
                    pend = None
                    emit_diff(32)
                    flush(0, avs)
                    # o_chain/epilogue work for finished thirds is deferred a
                    # few kt into the NEXT third so the in-order PE queue never
                    # stalls on flush copies / the pool-gather DMA.
                    for t3 in (1, 2):
                        avs = av_tiles(t3)
                        av_fn = lambda h, a=avs: a[h // 2][D * (h % 2):D * (h % 2) + DV1, :]
                        for kt in range(LT):
                            sc = scores_iter(t3, kt, psE)
                            if pend is not None:
                                av_iter(pend[0], pend[1], av_fn)
                            ex = exp_iter(sc, expp)
                            pend = (kt, ex)
                            if kt == 2:
                                o_chain(t3 - 1, tp_pool)
                            if t3 == 2 and kt == 6:
                                epilogue_half(0, tp_pool)
                        av_iter(pend[0], pend[1], av_fn)
                        pend = None
                        flush(t3, avs)

                # ================= tail (psE/psEa freed; tp_pool, pl alive) ==
                o_chain(2, tp_pool, ceng=nc.scalar)
                epilogue_half(1, tp_pool, ceng=nc.scalar)

    nc.finalize()
    return nc


def _prep_in_maps(inputs):
    import ml_dtypes
    bf = ml_dtypes.bfloat16

    q_x = np.asarray(inputs["q_x"], np.float32)
    v_x = np.asarray(inputs["v_x"], np.float32)
    q_adj = np.asarray(inputs["q_adj"], np.float32)
    v_adj = np.asarray(inputs["v_adj"], np.float32)
    w_qs = np.asarray(inputs["w_qs"], np.float32)
    w_ks = np.asarray(inputs["w_ks"], np.float32)
    w_vs = np.asarray(inputs["w_vs"], np.float32)
    w_fc = np.asarray(inputs["w_fc"], np.float32)
    mha_ln_w = np.asarray(inputs["mha_ln_w"], np.float32)
    mha_ln_b = np.asarray(inputs["mha_ln_b"], np.float32)
    conv_w = np.asarray(inputs["conv_w"], np.float32)
    conv_b = np.asarray(inputs["conv_b"], np.float32)
    lin_w = np.asarray(inputs["lin_w"], np.float32)
    lin_b = np.asarray(inputs["lin_b"], np.float32)
    norm_w = np.asarray(inputs["norm_w"], np.float32)
    norm_b = np.asarray(inputs["norm_b"], np.float32)

    # M_h = (Wq_h @ Wk_h^T) / sqrt(DK), stacked along columns
    m_all = np.zeros((D, H * D), np.float32)
    for h in range(H):
        m_all[:, D * h:D * (h + 1)] = (
            w_qs[:, DK * h:DK * (h + 1)] @ w_ks[:, DK * h:DK * (h + 1)].T
        ) / np.sqrt(DK)
    # augmented V projection: per head 16 value cols + a ones col (row 64)
    wv_aug = np.zeros((D + 1, H * DV1), np.float32)
    for h in range(H):
        wv_aug[:D, DV1 * h:DV1 * h + DV] = w_vs[:, DV * h:DV * (h + 1)]
        wv_aug[D, DV1 * h + DV] = 1.0
    conv_w3 = conv_w / 3.0

    shared = dict(
        m_bf=m_all.astype(bf),
        wv_aug=wv_aug.astype(bf),
        wfc=w_fc.astype(bf), mha_w=mha_ln_w, mha_b=mha_ln_b,
        conv_w3=conv_w3.astype(bf), conv_b=conv_b,
        lin_w=lin_w.astype(bf), lin_b=lin_b, norm_w=norm_w, norm_b=norm_b,
    )

    per_batch = []
    for b in range(B):
        A, Av = q_adj[b], v_adj[b]
        A2 = A @ A
        G = np.concatenate([np.eye(N, dtype=np.float32), A, A2], axis=0)  # [3N, N]
        per_batch.append(dict(
            xqb=q_x[b].astype(bf),
            xvb=v_x[b].astype(bf),
            atq=np.ascontiguousarray(A.T).astype(bf),
            atv=np.ascontiguousarray(Av.T).astype(bf),
            G=G,
        ))

    in_maps = []
    for c in range(8):
        b, g = c // 4, c % 4
        pb = per_batch[b]
        acq = np.ascontiguousarray(pb["G"][CH * g:CH * (g + 1)].T).astype(bf)  # [N, CH]
        rest = np.ascontiguousarray(q_x[b, CN * g:CN * (g + 1)].T)             # [D, CN]
        m = dict(shared)
        m.update(xqb=pb["xqb"], xvb=pb["xvb"],
                 atq=pb["atq"], atv=pb["atv"], acq=acq, rest=rest)
        in_maps.append(m)
    return in_maps


def _run(inputs, trace=False, **kw):
    from concourse.bass_utils import run_bass_kernel_spmd

    if "nc" not in _CACHE:
        _CACHE["nc"] = _build_nc()
    nc = _CACHE["nc"]
    in_maps = _prep_in_maps(inputs)
    res = run_bass_kernel_spmd(nc, in_maps, core_ids=list(range(8)), trace=trace, **kw)
    out = np.empty((B, N, D), np.float32)
    for c in range(8):
        b, g = c // 4, c % 4
        out[b, CN * g:CN * (g + 1)] = res.results[c]["out"]
    return out, res


def kernel(**inputs) -> np.ndarray:
    out, _ = _run(inputs, trace=False)
    return out
